# revision 7
# baseline (speedup 1.0000x reference)
"""Trainium2 Bass kernel for nn_MultiHeadAttention (B=2, S=2048, D=2048, H=16).

Sharding: tensor-parallel over heads -- each of the 8 cores owns 2 heads
(both batches) for the q/k/v projections and attention, then an 8-way
AllToAll converts the head-sharded attention output Y^T [256, 4096] into a
token-sharded layout [2048, 512], so each core computes a disjoint 512-token
slice of the output projection (no all-reduce needed).

Layout trick: all projections are computed with the contraction dim on SBUF
partitions, producing Q^T/K^T in [dh, t] layout directly (scores are computed
transposed: S^T[j,i] = sum_dh K^T[dh,j] Q^T[dh,i]) so no on-device transposes
are ever needed.  Softmax over keys j (the partition dim of S^T) is done
without max-subtraction (scores are O(1) here) via exp on ACT; the row sums
are an all-ones matmul on the tensor engine which lands pre-broadcast across
partitions; 1/sum via DVE reciprocal_approx_fast.

Host does only data marshalling: transposes (x^T, w^T slices), sharding, and
the final concat/transpose of per-core output slices.
"""

import os
import sys

import numpy as np

_REPO = "/opt/trn_rl_repo"
if _REPO not in sys.path:
    sys.path.insert(0, _REPO)

from concourse import bacc, mybir, tile  # noqa: E402
import concourse.bass as bass  # noqa: E402

B, S, D, H = 2, 2048, 2048, 16
DH = D // H  # 128
NCORES = 8
HPC = H // NCORES  # heads per core = 2
JW = HPC * DH  # per-core head-feature width = 256
T = B * S  # 4096 flattened tokens
TSL = T // NCORES  # per-core output token slice = 512
SCALE = float(np.sqrt(DH))

F32 = mybir.dt.float32
F32R = mybir.dt.float32r
AF = mybir.ActivationFunctionType
ALU = bass.mybir.AluOpType

P = 128
IT = 512  # query i-tile width
NIT = S // IT  # 4 i-tiles per (batch, head)
NJC = S // P  # 16 key chunks per batch


def _r(ap):
    return ap.bitcast(F32R)


def build_program():
    nc = bacc.Bacc(
        "TRN2",
        target_bir_lowering=False,
        debug=False,
        num_devices=NCORES,
    )

    # ---- kernel I/O (per-core values supplied via in_maps) ----
    xT = nc.dram_tensor("xT", [D, T], F32, kind="ExternalInput").ap()
    wqT = nc.dram_tensor("wqT", [D, JW], F32, kind="ExternalInput").ap()
    wkT = nc.dram_tensor("wkT", [D, JW], F32, kind="ExternalInput").ap()
    wvT = nc.dram_tensor("wvT", [D, JW], F32, kind="ExternalInput").ap()
    woT = nc.dram_tensor("woT", [D, D], F32, kind="ExternalInput").ap()
    bq = nc.dram_tensor("bq", [JW], F32, kind="ExternalInput").ap()
    bk = nc.dram_tensor("bk", [JW], F32, kind="ExternalInput").ap()
    bv = nc.dram_tensor("bv", [JW], F32, kind="ExternalInput").ap()
    bo = nc.dram_tensor("bo", [D], F32, kind="ExternalInput").ap()
    # 4 diagonal-band mask patterns (1.0 = attend), [m][jj][ii]
    maskp = nc.dram_tensor("maskp", [4, P, IT], F32, kind="ExternalInput").ap()
    ones = nc.dram_tensor("ones", [P, P], F32, kind="ExternalInput").ap()
    out = nc.dram_tensor("out", [D, TSL], F32, kind="ExternalOutput").ap()

    with tile.TileContext(nc) as tc:
        with (
            tc.tile_pool(name="dram", bufs=1, space="DRAM") as dram,
            tc.tile_pool(name="const", bufs=1) as cpool,
        ):
            # DRAM scratch
            qT_d = dram.tile([HPC, DH, T], F32)  # [head][dh][t]
            kT_d = dram.tile([HPC, DH, T], F32)
            v_d = dram.tile([T, JW], F32)  # [t][j]
            a2a_in = dram.tile([NCORES, JW, TSL], F32)
            a2a_out = dram.tile([D, TSL], F32)

            # constants
            mask_sb = cpool.tile([P, 4, IT], F32)
            nc.sync.dma_start(mask_sb[:], maskp.rearrange("m p i -> p m i"))
            ones_sb = cpool.tile([P, P], F32R)
            nc.sync.dma_start(ones_sb[:], ones.bitcast(F32R))
            bq_sb = cpool.tile([P, HPC], F32)
            nc.sync.dma_start(bq_sb[:], bq.rearrange("(h p) -> p h", p=P))
            bk_sb = cpool.tile([P, HPC], F32)
            nc.sync.dma_start(bk_sb[:], bk.rearrange("(h p) -> p h", p=P))
            bv_sb = cpool.tile([P, HPC], F32)
            nc.sync.dma_start(bv_sb[:], bv.rearrange("(h p) -> p h", p=P))
            bo_sb = cpool.tile([P, D // P], F32)
            nc.sync.dma_start(bo_sb[:], bo.rearrange("(e p) -> p e", p=P))

            xT_r = xT.rearrange("(dc p) t -> p dc t", p=P)
            NDC = D // P  # 16 contraction chunks

            # ---------- phase 1: q/k/v projections (to DRAM scratch) ----------
            with (
                tc.tile_pool(name="wpool", bufs=1) as wpool,
                tc.tile_pool(name="xpool", bufs=2) as xpool,
                tc.tile_pool(name="stage", bufs=3) as stage,
                tc.tile_pool(name="psum_p", bufs=3, space="PSUM") as psum_p,
            ):
                wq_sb = wpool.tile([P, NDC, JW], F32R, tag="wq")
                nc.sync.dma_start(wq_sb[:], wqT.rearrange("(dc p) j -> p dc j", p=P).bitcast(F32R))
                wk_sb = wpool.tile([P, NDC, JW], F32R, tag="wk")
                nc.sync.dma_start(wk_sb[:], wkT.rearrange("(dc p) j -> p dc j", p=P).bitcast(F32R))
                wv_sb = wpool.tile([P, NDC, JW], F32R, tag="wv")
                nc.sync.dma_start(wv_sb[:], wvT.rearrange("(dc p) j -> p dc j", p=P).bitcast(F32R))

                NTS = T // IT  # 8 token slices
                for ts in range(NTS):
                    x_sb = xpool.tile([P, NDC, IT], F32R, tag="x")
                    nc.sync.dma_start(x_sb[:], xT_r[:, :, ts * IT : (ts + 1) * IT].bitcast(F32R))
                    # Q^T and K^T: psum[j(dh of head h), t]
                    for w_sb, b_sb, out_d in ((wq_sb, bq_sb, qT_d), (wk_sb, bk_sb, kT_d)):
                        for h in range(HPC):
                            ps = psum_p.tile([P, IT], F32, tag="qk")
                            for dc in range(NDC):
                                nc.tensor.matmul(
                                    ps[:],
                                    lhsT=w_sb[:, dc, h * DH : (h + 1) * DH],
                                    rhs=x_sb[:, dc, :],
                                    start=(dc == 0),
                                    stop=(dc == NDC - 1),
                                )
                            st = stage.tile([P, IT], F32, tag="qkst")
                            nc.scalar.activation(
                                st[:], ps[:], AF.Identity, bias=b_sb[:, h : h + 1]
                            )
                            nc.sync.dma_start(
                                out_d[h, :, ts * IT : (ts + 1) * IT], st[:]
                            )
                    # V: psum[t-chunk, j] (natural layout; bias applied later)
                    for tc2 in range(IT // P):
                        ps = psum_p.tile([P, JW], F32, tag="v")
                        for dc in range(NDC):
                            nc.tensor.matmul(
                                ps[:],
                                lhsT=x_sb[:, dc, tc2 * P : (tc2 + 1) * P],
                                rhs=wv_sb[:, dc, :],
                                start=(dc == 0),
                                stop=(dc == NDC - 1),
                            )
                        stv = stage.tile([P, JW], F32, tag="vst")
                        nc.scalar.copy(stv[:], ps[:])
                        r0 = ts * IT + tc2 * P
                        nc.sync.dma_start(v_d[r0 : r0 + P, :], stv[:])

            # ---------- phase 2: attention (head-sharded, causal) ----------
            with tc.tile_pool(name="wo", bufs=1) as wopool:
                wo_sb = wopool.tile([P, NDC, D], F32R)
                nc.sync.dma_start(wo_sb[:], woT.rearrange("(jc p) e -> p jc e", p=P).bitcast(F32R))

                with (
                    tc.tile_pool(name="kv", bufs=1) as kvpool,
                    tc.tile_pool(name="small", bufs=2) as small,
                    tc.tile_pool(name="epool", bufs=2) as epool,
                    tc.tile_pool(name="psS", bufs=2, space="PSUM") as psS,
                    tc.tile_pool(name="psO", bufs=2, space="PSUM") as psO,
                    tc.tile_pool(name="psR", bufs=2, space="PSUM") as psR,
                ):
                    for b in range(B):
                        for lh in range(HPC):
                            kT_sb = kvpool.tile([P, S], F32R, tag="k")
                            nc.sync.dma_start(
                                kT_sb[:], kT_d[lh, :, b * S : (b + 1) * S].bitcast(F32R)
                            )
                            v_sb = kvpool.tile([P, NJC, DH], F32R, tag="v")
                            nc.sync.dma_start(
                                v_sb[:],
                                v_d[
                                    b * S : (b + 1) * S, lh * DH : (lh + 1) * DH
                                ].rearrange("(tc p) d -> p tc d", p=P).bitcast(F32R),
                            )
                            for it in range(NIT):
                                q_sb = small.tile([P, IT], F32R, tag="q")
                                t0 = b * S + it * IT
                                nc.sync.dma_start(q_sb[:], qT_d[lh, :, t0 : t0 + IT].bitcast(F32R))
                                njc = (it + 1) * (IT // P)  # causal: key chunks needed
                                po = psO.tile([P, IT], F32, tag="o")
                                pr = psR.tile([P, IT], F32, tag="r")
                                for jg in range(njc // 2):
                                    ps2 = psS.tile([P, 2, IT], F32, tag="s")
                                    for k2 in range(2):
                                        jc = jg * 2 + k2
                                        nc.tensor.matmul(
                                            ps2[:, k2, :],
                                            lhsT=kT_sb[:, jc * P : (jc + 1) * P],
                                            rhs=q_sb[:],
                                            start=True,
                                            stop=True,
                                        )
                                    e_sb = epool.tile([P, 2, IT], F32R, tag="e")
                                    nc.scalar.activation(
                                        e_sb[:], ps2[:], AF.Exp, scale=1.0 / SCALE
                                    )
                                    for k2 in range(2):
                                        jc = jg * 2 + k2
                                        if jc >= (it * IT) // P:
                                            m = jc - (it * IT) // P
                                            nc.vector.tensor_tensor(
                                                e_sb[:, k2, :],
                                                e_sb[:, k2, :],
                                                mask_sb[:, m, :],
                                                ALU.mult,
                                            )
                                        nc.tensor.matmul(
                                            po[:],
                                            lhsT=v_sb[:, jc, :],
                                            rhs=e_sb[:, k2, :],
                                            start=(jc == 0),
                                            stop=(jc == njc - 1),
                                        )
                                        nc.tensor.matmul(
                                            pr[:],
                                            lhsT=ones_sb[:],
                                            rhs=e_sb[:, k2, :],
                                            start=(jc == 0),
                                            stop=(jc == njc - 1),
                                        )
                                rinv = small.tile([P, IT], F32, tag="rinv")
                                nc.vector.reciprocal_approx_fast(rinv[:], pr[:])
                                y_sb = small.tile([P, IT], F32, tag="y")
                                nc.vector.tensor_tensor(
                                    y_sb[:], po[:], rinv[:], ALU.mult
                                )
                                nc.vector.tensor_tensor(
                                    y_sb[:],
                                    y_sb[:],
                                    bv_sb[:, lh : lh + 1].to_broadcast([P, IT]),
                                    ALU.add,
                                )
                                g = NIT * b + it  # destination core / a2a block
                                nc.sync.dma_start(
                                    a2a_in[g, lh * DH : (lh + 1) * DH, :], y_sb[:]
                                )

                # ---------- all-to-all: head-sharded -> token-sharded ----------
                nc.gpsimd.collective_compute(
                    "AllToAll",
                    ALU.bypass,
                    replica_groups=[list(range(NCORES))],
                    ins=[a2a_in[:].opt()],
                    outs=[a2a_out[:].opt()],
                )

                # ---------- phase 3: output projection on own token slice ----------
                with (
                    tc.tile_pool(name="ya", bufs=1) as yapool,
                    tc.tile_pool(name="ostage", bufs=3) as ostage,
                    tc.tile_pool(name="psout", bufs=4, space="PSUM") as psout,
                ):
                    ya_sb = yapool.tile([P, NDC, TSL], F32R)
                    nc.sync.dma_start(
                        ya_sb[:], a2a_out[:].rearrange("(jc p) i -> p jc i", p=P).bitcast(F32R)
                    )
                    for ec in range(D // P):
                        ps = psout.tile([P, TSL], F32, tag="out")
                        for jc in range(NDC):
                            nc.tensor.matmul(
                                ps[:],
                                lhsT=wo_sb[:, jc, ec * P : (ec + 1) * P],
                                rhs=ya_sb[:, jc, :],
                                start=(jc == 0),
                                stop=(jc == NDC - 1),
                            )
                        ost = ostage.tile([P, TSL], F32, tag="ost")
                        nc.scalar.activation(
                            ost[:], ps[:], AF.Identity, bias=bo_sb[:, ec : ec + 1]
                        )
                        nc.sync.dma_start(out[ec * P : (ec + 1) * P, :], ost[:])

    nc.finalize()  # bacc compile: regalloc etc. -- required before execution
    return nc


_PROGRAM = None


def _get_program():
    global _PROGRAM
    if _PROGRAM is None:
        _PROGRAM = build_program()
    return _PROGRAM


def _host_prep(x, mask, wq, bq, wk, bk, wv, bv, wo, bo):
    """Build the 8 per-core input maps (host-side marshalling only)."""
    f = np.float32
    x2 = np.asarray(x, dtype=f).reshape(T, D)
    xT = np.ascontiguousarray(x2.T)  # [D, T]
    woT = np.ascontiguousarray(np.asarray(wo, dtype=f).T)  # [D, D]
    bo_ = np.ascontiguousarray(np.asarray(bo, dtype=f))

    # diagonal-band mask patterns from the provided mask (True = masked out).
    # For diag chunk offset m (0..3): pattern[jj, ii] applies to key j = m*128+jj,
    # query i = ii within a 512-wide i-tile starting at the same 512 boundary.
    mask_np = np.asarray(mask)
    maskp = np.empty((4, P, IT), dtype=f)
    for m in range(4):
        # allowed = not masked; mask[i, j] with i = query, j = key
        maskp[m] = (~mask_np[0:IT, m * P : (m + 1) * P]).T.astype(f)
    maskp = np.ascontiguousarray(maskp)

    wq_, wk_, wv_ = (np.asarray(w, dtype=f) for w in (wq, wk, wv))
    bq_, bk_, bv_ = (np.asarray(v_, dtype=f) for v_ in (bq, bk, bv))

    in_maps = []
    for c in range(NCORES):
        j0, j1 = c * JW, (c + 1) * JW
        in_maps.append(
            {
                "xT": xT,
                "wqT": np.ascontiguousarray(wq_[j0:j1, :].T),
                "wkT": np.ascontiguousarray(wk_[j0:j1, :].T),
                "wvT": np.ascontiguousarray(wv_[j0:j1, :].T),
                "woT": woT,
                "bq": np.ascontiguousarray(bq_[j0:j1]),
                "bk": np.ascontiguousarray(bk_[j0:j1]),
                "bv": np.ascontiguousarray(bv_[j0:j1]),
                "bo": bo_,
                "maskp": maskp,
                "ones": np.ones((P, P), dtype=f),
            }
        )
    return in_maps


LAST_RESULTS = None  # BassKernelResults of the most recent run (for test.py)


def kernel(x, mask, wq, bq, wk, bk, wv, bv, wo, bo):
    global LAST_RESULTS
    from concourse.bass_utils import run_bass_kernel_spmd

    nc = _get_program()
    in_maps = _host_prep(x, mask, wq, bq, wk, bk, wv, bv, wo, bo)
    trace = os.environ.get("KERNEL_TRACE", "") == "1"
    res = run_bass_kernel_spmd(
        nc, in_maps, core_ids=list(range(NCORES)), trace=trace
    )
    LAST_RESULTS = res
    # assemble: per-core out is out^T slice [D, 512]; concat on token axis,
    # transpose back to [T, D], reshape to [B, S, D]
    outT = np.concatenate([res.results[c]["out"] for c in range(NCORES)], axis=1)
    return np.ascontiguousarray(outT.T).reshape(B, S, D).astype(np.float32)
